# revision 13
# baseline (speedup 1.0000x reference)
"""LocalCorrelation (13x13 cost volume) Trainium2 kernel, v2.

Full inputs z_t, z_t1: [8, 256, 128, 128] f32 -> out [8, 169, 128, 128] f32.
out[b, 13*di+dj, h, w] = sum_c z_t[b,c,h,w] * pad(z_t1)[b,c,h+di,w+dj] / 16

Sharding: data-parallel over batch, 1 batch element per NeuronCore (8 cores).

Per-core v2 pipeline (all on-chip, no DRAM scratch), software-pipelined
two stripes deep so TensorE streams back-to-back (p-state ramp):
  stage A (stripe si):   block-gram matmuls -> PSUM -> xb (bf16);
                         hop1: 16 band DMAs (SBUF->SBUF, absorb the
                         per-row dh*20 window shear, repartition
                         p=dh*8+dw -> p2=dw*16+dh);
                         hop2: 8 DMAs (absorb the per-col +dw shear,
                         constant offset per dw group).
  stage B (stripe si-1): one strided copy extracts the 169 taps/pixel,
                         TensorE perm-matmul transposes taps onto
                         partitions (with 1/16 scale), obuf assembly,
                         output DMAs with 2KB runs.
"""

import numpy as np

C = 256
H = W = 128
KS = 13
KK = 169
RAD = 6
HP = WP = 140          # padded spatial
SA = 16                # stripe rows
SB = 8                 # block cols
NST = H // SA          # 8 stripes
NWB = W // SB          # 16 w-blocks
WINP = SA + 2 * RAD    # 28 window rows
WINQ = SB + 2 * RAD    # 20 window cols
WIN = WINP * WINQ      # 560
BAND = 260             # 12*20 + 12 + 8: per-pixel tap band (+dw slack)
EB = 253               # dw-aligned band (12*20 + 12 + 1)
NE = NWB * BAND        # 4160 o5 elems / partition
NEB = NWB * EB         # 4048 o5b elems / partition
NI2 = 2880             # o6 free, padded past 2704 for lhsB reads
TCA = 117              # tap chunk A size (and padded B stationary width)
TCB = KK - TCA         # 52 real taps in chunk B
FS = NWB * WIN         # 8960 xb free size

_cache = {}


def _consts():
    # permutation matrix: p2 = dw*16+dh -> pixel n = dh*8+dw, with 1/16 scale
    perm = np.zeros((128, 128), np.float32)
    for p2 in range(128):
        dw, dh = divmod(p2, 16)
        perm[p2, dh * 8 + dw] = 1.0 / 16.0
    return perm


def _build():
    import concourse.bass as bass
    import concourse.mybir as mybir
    import concourse.tile as tile
    from concourse import bacc

    f32 = mybir.dt.float32
    bf16 = mybir.dt.bfloat16

    nc = bacc.Bacc("TRN2", target_bir_lowering=False, debug=False)
    zt_d = nc.dram_tensor("z_t", [C, H, W], f32, kind="ExternalInput")
    z1_d = nc.dram_tensor("z_t1", [C, H, W], f32, kind="ExternalInput")
    perm_d = nc.dram_tensor("perm", [128, 128], f32, kind="ExternalInput")
    out_d = nc.dram_tensor("out", [KK, H, W], f32, kind="ExternalOutput")

    with tile.TileContext(nc) as tc:
        with tc.tile_pool(name="persist", bufs=1) as pp:
            Z1P = [pp.tile([128, HP * WP], bf16, tag=f"z1p{k}", name=f"z1p{k}")
                   for k in range(2)]
            permf = pp.tile([128, 128], f32, tag="permf", name="permf")
            perm = pp.tile([128, 128], bf16, tag="perm", name="perm")

            nc.sync.dma_start(permf[:, :], perm_d.ap()[:, :])
            nc.vector.tensor_copy(perm[:, :], permf[:, :])

            # z1 padded halo memsets (top/bottom rows, left/right cols)
            for k in range(2):
                zv = Z1P[k].rearrange("c (h w) -> c h w", h=HP)
                nc.vector.memset(zv[:, 0:RAD, :], 0.0)
                nc.vector.memset(zv[:, HP - RAD:HP, :], 0.0)
                nc.vector.memset(zv[:, RAD:HP - RAD, 0:RAD], 0.0)
                nc.vector.memset(zv[:, RAD:HP - RAD, WP - RAD:WP], 0.0)
            # z1 interior load with f32->bf16 cast (gpsimd swdge)
            def load_z1():
                for k in range(2):
                    zv = Z1P[k].rearrange("c (h w) -> c h w", h=HP)
                    for s in range(4):
                        sap = z1_d.ap()[k * 128:(k + 1) * 128,
                                        s * 32:(s + 1) * 32, :]
                        dst = zv[:, RAD + s * 32: RAD + (s + 1) * 32,
                                 RAD: RAD + W]
                        nc.gpsimd.dma_start(dst, sap)

            with (
                tc.tile_pool(name="ztup", bufs=1) as ztup,
                tc.tile_pool(name="ztp", bufs=2) as ztp,
                tc.tile_pool(name="xbp", bufs=3) as xbp,
                tc.tile_pool(name="o5p", bufs=1) as o5p,
                tc.tile_pool(name="o5bp", bufs=3) as o5bp,
                tc.tile_pool(name="o6p", bufs=1) as o6p,
                tc.tile_pool(name="obp", bufs=1) as obp,
                tc.tile_pool(name="psp", bufs=3, space="PSUM") as psp,
                tc.tile_pool(name="ptp", bufs=2, space="PSUM") as ptp,
            ):
                ztb = {}
                o5bs = {}

                def load_zt_stripe(s):
                    # DMA 16-row slab (cast), then rearrange to block-major:
                    # free = wb*128 + dh*8 + dw so each block's stationary
                    # operand is one contiguous 128-elem free dim.
                    t = [ztp.tile([128, SA * W], bf16, tag=f"ztb{k}",
                                  name=f"ztb{k}_{s}") for k in range(2)]
                    for k in range(2):
                        ztu = ztup.tile([128, SA * W], bf16, tag="ztu",
                                        name=f"ztu{k}_{s}")
                        src = zt_d.ap()[k * 128:(k + 1) * 128,
                                        s * SA:(s + 1) * SA, :]
                        nc.gpsimd.dma_start(
                            ztu.rearrange("c (h w) -> c h w", h=SA), src)
                        srcv = bass.AP(ztu.tensor, 0,
                                       [[SA * W, 128], [8, NWB], [W, SA], [1, SB]])
                        dstv = bass.AP(t[k].tensor, 0,
                                       [[SA * W, 128], [128, NWB], [SB, SA], [1, SB]])
                        if k == 0:
                            nc.vector.tensor_copy(dstv, srcv)
                        else:
                            nc.scalar.copy(dstv, srcv)
                    ztb[s] = t

                def stage_a(si):
                    """main matmuls + psum->xb + band hop1/hop2 DMAs"""
                    xb = xbp.tile([128, FS], bf16, tag="xb", name="xb")
                    for wb in range(NWB):
                        ps = psp.tile([128, 1024], f32, tag="ps", name="ps")
                        for k in range(2):
                            lhsT = ztb[si][k][:, wb * 128:(wb + 1) * 128]
                            for half in range(2):
                                rhs = Z1P[k].rearrange(
                                    "c (h w) -> c h w", h=HP)[
                                    :, si * SA + 14 * half: si * SA + 14 * (half + 1),
                                    wb * SB: wb * SB + WINQ]
                                nc.tensor.matmul(
                                    ps[:, half * 512: half * 512 + 280],
                                    lhsT, rhs, start=(k == 0), stop=(k == 1))
                        src = bass.AP(ps.tensor, 0,
                                      [[1024, 128], [512, 2], [1, 280]])
                        dst = bass.AP(xb.tensor, wb * WIN,
                                      [[FS, 128], [280, 2], [1, 280]])
                        if wb % 2 == 0:
                            nc.scalar.copy(dst, src)
                        else:
                            nc.vector.tensor_copy(dst, src)

                    # hop1: band + repartition: p = dh*8+dw -> p2 = dw*16+dh
                    o5 = o5p.tile([128, NE], bf16, tag="o5", name="o5")
                    for dh in range(SA):
                        src_ap = bass.AP(xb.tensor, dh * 8 * FS + dh * WINQ,
                                         [[FS, 8], [WIN, NWB], [1, BAND]])
                        dst_ap = bass.AP(o5.tensor, dh * NE,
                                         [[16 * NE, 8], [BAND, NWB], [1, BAND]])
                        nc.sync.dma_start(dst_ap, src_ap)
                    # hop2: per dw group, shift band start by dw
                    o5b = o5bp.tile([128, NEB], bf16, tag="o5b", name="o5b")
                    for dw in range(8):
                        src_ap = bass.AP(o5.tensor, dw * 16 * NE + dw,
                                         [[NE, 16], [BAND, NWB], [1, EB]])
                        dst_ap = bass.AP(o5b.tensor, dw * 16 * NEB,
                                         [[NEB, 16], [EB, NWB], [1, EB]])
                        nc.gpsimd.dma_start(dst_ap, src_ap)
                    o5bs[si] = o5b

                def stage_b(si):
                    """tap extraction + transpose + output for stripe si"""
                    o5b = o5bs.pop(si)
                    o6 = o6p.tile([128, NI2], bf16, tag="o6", name="o6")
                    # tail reads past 2704 hit stale data; the extra psum_t
                    # rows land in ob rows >= TCB that the output never reads
                    src = bass.AP(o5b.tensor, 0,
                                  [[NEB, 128], [EB, NWB], [WINQ, KS], [1, KS]])
                    dst = bass.AP(o6.tensor, 0,
                                  [[NI2, 128], [KK, NWB], [KS, KS], [1, KS]])
                    nc.vector.tensor_copy(dst, src)

                    # tap transpose: psum_t[t, dh*8+dw] via perm matmul
                    ob = obp.tile([128, 2 * SA * W], f32, tag="ob", name="ob")
                    for wb in range(NWB):
                        pt = ptp.tile([128, 256], f32, tag="pt", name="pt")
                        lhsA = bass.AP(o6.tensor, wb * KK,
                                       [[NI2, 128], [1, TCA]])
                        lhsB = bass.AP(o6.tensor, wb * KK + TCA,
                                       [[NI2, 128], [1, TCA]])
                        nc.tensor.matmul(pt[:TCA, 0:128], lhsA, perm[:, :],
                                         start=True, stop=True)
                        nc.tensor.matmul(pt[:TCA, 128:256], lhsB, perm[:, :],
                                         start=True, stop=True)
                        src = bass.AP(pt.tensor, 0,
                                      [[256, TCA], [128, 2], [8, SA], [1, SB]])
                        dst = bass.AP(ob.tensor, wb * SB,
                                      [[2 * SA * W, TCA], [SA * W, 2],
                                       [W, SA], [1, SB]])
                        if wb % 2 == 0:
                            nc.vector.tensor_copy(dst, src)
                        else:
                            nc.scalar.copy(dst, src)

                    # output: 8 DMAs (4 dh-quarters x 2 tap chunks), 2KB runs
                    for i, (tc_n, tbase, obase) in enumerate(
                            ((TCA, 0, 0), (TCB, TCA, SA * W))):
                        for dq in range(4):
                            src = bass.AP(ob.tensor, obase + dq * 4 * W,
                                          [[2 * SA * W, tc_n], [1, 4 * W]])
                            dst = bass.AP(out_d,
                                          tbase * H * W + (si * SA + dq * 4) * W,
                                          [[H * W, tc_n], [1, 4 * W]])
                            if dq % 2 == 0:
                                nc.sync.dma_start(dst, src)
                            else:
                                nc.scalar.dma_start(dst, src)

                load_zt_stripe(0)
                load_z1()
                for si in range(NST):
                    if si + 1 < NST:
                        load_zt_stripe(si + 1)
                    if si > 1:
                        stage_b(si - 2)
                    stage_a(si)
                stage_b(NST - 2)
                stage_b(NST - 1)

    nc.compile()
    return nc


def _get_nc():
    if "nc" not in _cache:
        _cache["nc"] = _build()
    return _cache["nc"]


def kernel(z_t: np.ndarray, z_t1: np.ndarray) -> np.ndarray:
    from concourse.bass_utils import run_bass_kernel_spmd

    nc = _get_nc()
    z_t = np.ascontiguousarray(z_t, dtype=np.float32)
    z_t1 = np.ascontiguousarray(z_t1, dtype=np.float32)
    perm_f = _consts()
    B = z_t.shape[0]
    in_maps = [{"z_t": z_t[i], "z_t1": z_t1[i], "perm": perm_f}
               for i in range(B)]
    res = run_bass_kernel_spmd(nc, in_maps, core_ids=list(range(B)))
    return np.stack([res.results[i]["out"] for i in range(B)], axis=0)


# revision 14
# speedup vs baseline: 1.2224x; 1.2224x over previous
"""LocalCorrelation (13x13 cost volume) Trainium2 kernel, v2.

Full inputs z_t, z_t1: [8, 256, 128, 128] f32 -> out [8, 169, 128, 128] f32.
out[b, 13*di+dj, h, w] = sum_c z_t[b,c,h,w] * pad(z_t1)[b,c,h+di,w+dj] / 16

Sharding: data-parallel over batch, 1 batch element per NeuronCore (8 cores).

Per-core v2 pipeline (all on-chip, no DRAM scratch), software-pipelined
two stripes deep so TensorE streams back-to-back (p-state ramp):
  stage A (stripe si):   block-gram matmuls -> PSUM -> xb (bf16);
                         hop1: 16 band DMAs (SBUF->SBUF, absorb the
                         per-row dh*20 window shear, repartition
                         p=dh*8+dw -> p2=dw*16+dh);
                         hop2: 8 DMAs (absorb the per-col +dw shear,
                         constant offset per dw group).
  stage B (stripe si-1): one strided copy extracts the 169 taps/pixel,
                         TensorE perm-matmul transposes taps onto
                         partitions (with 1/16 scale), obuf assembly,
                         output DMAs with 2KB runs.
"""

import numpy as np

C = 256
H = W = 128
KS = 13
KK = 169
RAD = 6
HP = WP = 140          # padded spatial
SA = 16                # stripe rows
SB = 8                 # block cols
NST = H // SA          # 8 stripes
NWB = W // SB          # 16 w-blocks
WINP = SA + 2 * RAD    # 28 window rows
WINQ = SB + 2 * RAD    # 20 window cols
WIN = WINP * WINQ      # 560
BAND = 260             # 12*20 + 12 + 8: per-pixel tap band (+dw slack)
EB = 253               # dw-aligned band (12*20 + 12 + 1)
NE = NWB * BAND        # 4160 o5 elems / partition
NEB = NWB * EB         # 4048 o5b elems / partition
NI2 = 2880             # o6 free, padded past 2704 for lhsB reads
TCA = 117              # tap chunk A size (and padded B stationary width)
TCB = KK - TCA         # 52 real taps in chunk B
FS = NWB * WIN         # 8960 xb free size

_cache = {}


def _consts():
    # permutation matrix: p2 = dw*16+dh -> pixel n = dh*8+dw, with 1/16 scale
    perm = np.zeros((128, 128), np.float32)
    for p2 in range(128):
        dw, dh = divmod(p2, 16)
        perm[p2, dh * 8 + dw] = 1.0 / 16.0
    return perm


def _build():
    import concourse.bass as bass
    import concourse.mybir as mybir
    import concourse.tile as tile
    from concourse import bacc

    f32 = mybir.dt.float32
    bf16 = mybir.dt.bfloat16

    nc = bacc.Bacc("TRN2", target_bir_lowering=False, debug=False)
    zt_d = nc.dram_tensor("z_t", [C, H, W], f32, kind="ExternalInput")
    z1_d = nc.dram_tensor("z_t1", [C, H, W], f32, kind="ExternalInput")
    perm_d = nc.dram_tensor("perm", [128, 128], f32, kind="ExternalInput")
    out_d = nc.dram_tensor("out", [KK, H, W], f32, kind="ExternalOutput")

    with tile.TileContext(nc) as tc:
        with tc.tile_pool(name="persist", bufs=1) as pp:
            Z1P = [pp.tile([128, HP * WP], bf16, tag=f"z1p{k}", name=f"z1p{k}")
                   for k in range(2)]
            permf = pp.tile([128, 128], f32, tag="permf", name="permf")
            perm = pp.tile([128, 128], bf16, tag="perm", name="perm")

            nc.sync.dma_start(permf[:, :], perm_d.ap()[:, :])
            nc.vector.tensor_copy(perm[:, :], permf[:, :])

            # z1 padded halo memsets (top/bottom rows, left/right cols)
            for k in range(2):
                zv = Z1P[k].rearrange("c (h w) -> c h w", h=HP)
                nc.vector.memset(zv[:, 0:RAD, :], 0.0)
                nc.vector.memset(zv[:, HP - RAD:HP, :], 0.0)
                nc.vector.memset(zv[:, RAD:HP - RAD, 0:RAD], 0.0)
                nc.vector.memset(zv[:, RAD:HP - RAD, WP - RAD:WP], 0.0)
            # z1 interior load with f32->bf16 cast (gpsimd swdge)
            def load_z1():
                for k in range(2):
                    zv = Z1P[k].rearrange("c (h w) -> c h w", h=HP)
                    for s in range(4):
                        sap = z1_d.ap()[k * 128:(k + 1) * 128,
                                        s * 32:(s + 1) * 32, :]
                        dst = zv[:, RAD + s * 32: RAD + (s + 1) * 32,
                                 RAD: RAD + W]
                        nc.gpsimd.dma_start(dst, sap)

            with (
                tc.tile_pool(name="ztup", bufs=2) as ztup,
                tc.tile_pool(name="ztp", bufs=2) as ztp,
                tc.tile_pool(name="xbp", bufs=2) as xbp,
                tc.tile_pool(name="o5p", bufs=2) as o5p,
                tc.tile_pool(name="o5bp", bufs=3) as o5bp,
                tc.tile_pool(name="o6p", bufs=2) as o6p,
                tc.tile_pool(name="obp", bufs=1) as obp,
                tc.tile_pool(name="psp", bufs=3, space="PSUM") as psp,
                tc.tile_pool(name="ptp", bufs=2, space="PSUM") as ptp,
            ):
                ztb = {}
                o5bs = {}

                def load_zt_stripe(s):
                    # DMA 16-row slab (cast), then rearrange to block-major:
                    # free = wb*128 + dh*8 + dw so each block's stationary
                    # operand is one contiguous 128-elem free dim.
                    t = [ztp.tile([128, SA * W], bf16, tag=f"ztb{k}",
                                  name=f"ztb{k}_{s}") for k in range(2)]
                    for k in range(2):
                        ztu = ztup.tile([128, SA * W], bf16, tag="ztu",
                                        name=f"ztu{k}_{s}")
                        src = zt_d.ap()[k * 128:(k + 1) * 128,
                                        s * SA:(s + 1) * SA, :]
                        nc.gpsimd.dma_start(
                            ztu.rearrange("c (h w) -> c h w", h=SA), src)
                        srcv = bass.AP(ztu.tensor, 0,
                                       [[SA * W, 128], [8, NWB], [W, SA], [1, SB]])
                        dstv = bass.AP(t[k].tensor, 0,
                                       [[SA * W, 128], [128, NWB], [SB, SA], [1, SB]])
                        if k == 0:
                            nc.vector.tensor_copy(dstv, srcv)
                        else:
                            nc.scalar.copy(dstv, srcv)
                    ztb[s] = t

                def stage_a(si):
                    """main matmuls + psum->xb + band hop1/hop2 DMAs"""
                    xb = xbp.tile([128, FS], bf16, tag="xb", name="xb")
                    for wb in range(NWB):
                        ps = psp.tile([128, 1024], f32, tag="ps", name="ps")
                        for k in range(2):
                            lhsT = ztb[si][k][:, wb * 128:(wb + 1) * 128]
                            for half in range(2):
                                rhs = Z1P[k].rearrange(
                                    "c (h w) -> c h w", h=HP)[
                                    :, si * SA + 14 * half: si * SA + 14 * (half + 1),
                                    wb * SB: wb * SB + WINQ]
                                nc.tensor.matmul(
                                    ps[:, half * 512: half * 512 + 280],
                                    lhsT, rhs, start=(k == 0), stop=(k == 1))
                        src = bass.AP(ps.tensor, 0,
                                      [[1024, 128], [512, 2], [1, 280]])
                        dst = bass.AP(xb.tensor, wb * WIN,
                                      [[FS, 128], [280, 2], [1, 280]])
                        if wb % 2 == 0:
                            nc.scalar.copy(dst, src)
                        else:
                            nc.vector.tensor_copy(dst, src)

                    # hop1: band + repartition: p = dh*8+dw -> p2 = dw*16+dh
                    o5 = o5p.tile([128, NE], bf16, tag="o5", name="o5")
                    for dh in range(SA):
                        src_ap = bass.AP(xb.tensor, dh * 8 * FS + dh * WINQ,
                                         [[FS, 8], [WIN, NWB], [1, BAND]])
                        dst_ap = bass.AP(o5.tensor, dh * NE,
                                         [[16 * NE, 8], [BAND, NWB], [1, BAND]])
                        nc.sync.dma_start(dst_ap, src_ap)
                    # hop2: per dw group, shift band start by dw
                    o5b = o5bp.tile([128, NEB], bf16, tag="o5b", name="o5b")
                    for dw in range(8):
                        src_ap = bass.AP(o5.tensor, dw * 16 * NE + dw,
                                         [[NE, 16], [BAND, NWB], [1, EB]])
                        dst_ap = bass.AP(o5b.tensor, dw * 16 * NEB,
                                         [[NEB, 16], [EB, NWB], [1, EB]])
                        nc.gpsimd.dma_start(dst_ap, src_ap)
                    o5bs[si] = o5b

                def stage_b(si):
                    """tap extraction + transpose + output for stripe si"""
                    o5b = o5bs.pop(si)
                    o6 = o6p.tile([128, NI2], bf16, tag="o6", name="o6")
                    # tail reads past 2704 hit stale data; the extra psum_t
                    # rows land in ob rows >= TCB that the output never reads
                    src = bass.AP(o5b.tensor, 0,
                                  [[NEB, 128], [EB, NWB], [WINQ, KS], [1, KS]])
                    dst = bass.AP(o6.tensor, 0,
                                  [[NI2, 128], [KK, NWB], [KS, KS], [1, KS]])
                    nc.vector.tensor_copy(dst, src)

                    # tap transpose: psum_t[t, dh*8+dw] via perm matmul
                    ob = obp.tile([128, 2 * SA * W], f32, tag="ob", name="ob")
                    for wb in range(NWB):
                        pt = ptp.tile([128, 256], f32, tag="pt", name="pt")
                        lhsA = bass.AP(o6.tensor, wb * KK,
                                       [[NI2, 128], [1, TCA]])
                        lhsB = bass.AP(o6.tensor, wb * KK + TCA,
                                       [[NI2, 128], [1, TCA]])
                        nc.tensor.matmul(pt[:TCA, 0:128], lhsA, perm[:, :],
                                         start=True, stop=True)
                        nc.tensor.matmul(pt[:TCA, 128:256], lhsB, perm[:, :],
                                         start=True, stop=True)
                        src = bass.AP(pt.tensor, 0,
                                      [[256, TCA], [128, 2], [8, SA], [1, SB]])
                        dst = bass.AP(ob.tensor, wb * SB,
                                      [[2 * SA * W, TCA], [SA * W, 2],
                                       [W, SA], [1, SB]])
                        if wb % 2 == 0:
                            nc.vector.tensor_copy(dst, src)
                        else:
                            nc.scalar.copy(dst, src)

                    # output: 8 DMAs (4 dh-quarters x 2 tap chunks), 2KB runs
                    for i, (tc_n, tbase, obase) in enumerate(
                            ((TCA, 0, 0), (TCB, TCA, SA * W))):
                        for dq in range(4):
                            src = bass.AP(ob.tensor, obase + dq * 4 * W,
                                          [[2 * SA * W, tc_n], [1, 4 * W]])
                            dst = bass.AP(out_d,
                                          tbase * H * W + (si * SA + dq * 4) * W,
                                          [[H * W, tc_n], [1, 4 * W]])
                            if dq % 2 == 0:
                                nc.sync.dma_start(dst, src)
                            else:
                                nc.scalar.dma_start(dst, src)

                load_zt_stripe(0)
                load_z1()
                for si in range(NST):
                    if si + 1 < NST:
                        load_zt_stripe(si + 1)
                    if si > 1:
                        stage_b(si - 2)
                    stage_a(si)
                stage_b(NST - 2)
                stage_b(NST - 1)

    nc.compile()
    return nc


def _get_nc():
    if "nc" not in _cache:
        _cache["nc"] = _build()
    return _cache["nc"]


def kernel(z_t: np.ndarray, z_t1: np.ndarray) -> np.ndarray:
    from concourse.bass_utils import run_bass_kernel_spmd

    nc = _get_nc()
    z_t = np.ascontiguousarray(z_t, dtype=np.float32)
    z_t1 = np.ascontiguousarray(z_t1, dtype=np.float32)
    perm_f = _consts()
    B = z_t.shape[0]
    in_maps = [{"z_t": z_t[i], "z_t1": z_t1[i], "perm": perm_f}
               for i in range(B)]
    res = run_bass_kernel_spmd(nc, in_maps, core_ids=list(range(B)))
    return np.stack([res.results[i]["out"] for i in range(B)], axis=0)
